# revision 21
# baseline (speedup 1.0000x reference)
"""GAT layer kernel for Trainium2, 8 NeuronCores — gather-free design.

Strategy (src-range sharding, no collectives, no indirect DMA):
  - Host: degree-balanced (LPT) assignment of nodes to (core, tile, slot);
    edges grouped by src tile into C chunks of 128 edge slots.  For every
    edge slot the host lays out a 264-col f16 row:
        [x[dst] (256) | e0 | e1 | srcL | pad(5)]
    where e_h = s_h[src] + t_h[dst] are the pre-activation attention
    scores (s = x@(W_h a_src), t = x@(W_h a_dst) — thin [N,4] matvec done
    host-side) and srcL is the edge's src slot within the tile.  Rows are
    stored partition-major ([tile, p, c, col]) so each tile loads with one
    128-descriptor sequential DMA.
  - Device, per tile: build one-hot oneh[e, s] = (srcL[e] == s) on DVE;
    p = exp(lrelu(e) - SHIFT) on Act; poneh_h = p_h * oneh.  Aggregate in
    x-space, transposed: zT[x, s] += xe_chunk(lhsT) @ poneh_h — 4 matmuls
    of 128 cols per chunk (2 heads x 2 x-halves) plus a 2-col matmul for
    the softmax denominators (lhsT = oneh, rhs = p).  Then one final
    out[s, :] = zT(lhsT) @ W per tile (the linear W factors out of the
    alpha-weighted sum), normalized by 1/den during PSUM copy-out.
    Deg-0 fallback (out = Wh[own]) via a per-tile gated matmul from xT.
"""

import math
import sys
from dataclasses import dataclass

import numpy as np

sys.path.insert(0, "/opt/trn_rl_repo")

import concourse.bass as bass
import concourse.mybir as mybir
import concourse.tile as tile
from concourse import bacc
from concourse.bass_utils import run_bass_kernel_spmd

# Problem shapes (fixed by the graded problem)
N_NODES = 50000
IN_DIM = 256
OUT_DIM = 128
NUM_HEADS = 2

P = 128
EW = 264  # edge row: x[dst](256), e0, e1, srcL, pad(5)
SHIFT = 4.0  # constant subtracted inside exp (softmax-invariant)

F32 = mybir.dt.float32
F16 = mybir.dt.float16
I32 = mybir.dt.int32


@dataclass(frozen=True)
class Cfg:
    n_nodes: int
    n_cores: int
    C: int
    deg0_tiles: tuple = ()  # (core, tile) pairs containing deg-0 nodes
    reps: int = 1

    @property
    def nodes_per_core(self):
        return self.n_nodes // self.n_cores

    @property
    def ntiles(self):
        return (self.nodes_per_core + P - 1) // P

    @property
    def npad(self):
        return self.n_cores * self.ntiles * P


def _ap_expand(ap, dims):
    """Return an AP keeping ap's partition dim and replacing the free dims
    with `dims` = list of (step, count) pairs (element units)."""
    return bass.AP(ap.tensor, ap.offset, [list(ap.ap[0])] + [[s, c] for s, c in dims])


def host_prep(x, edge_index, W_w, W_b, a, n_cores=8):
    """Index/layout preprocessing + parameter folding + score projections.
    Returns (cfg, shared_inputs, per_core_inputs)."""
    x = np.asarray(x, dtype=np.float32)
    edge_index = np.asarray(edge_index)
    W_w = np.asarray(W_w, dtype=np.float32)
    W_b = np.asarray(W_b, dtype=np.float32)
    a = np.asarray(a, dtype=np.float32)
    assert np.abs(W_b).max() == 0.0, "nonzero bias not supported"

    n_nodes, in_dim = x.shape
    D = OUT_DIM
    n_edges = edge_index.shape[1]

    # Parameter folding: per-head score vectors, then thin score projections.
    a_src, a_dst = a[:D], a[D:]
    wst = np.stack(
        [W_w[:, 0:D] @ a_src, W_w[:, D:] @ a_src,
         W_w[:, 0:D] @ a_dst, W_w[:, D:] @ a_dst], axis=1
    )  # [in_dim, 4]
    st = x @ wst  # [N, 4]: s0, s1, t0, t1

    src = np.asarray(edge_index[0], dtype=np.int64)
    dst = np.asarray(edge_index[1], dtype=np.int64)

    npc = n_nodes // n_cores
    ntiles = (npc + P - 1) // P

    # Degree-balanced global node->(core,tile,slot) assignment (LPT over
    # all tiles): equalizes per-tile edge counts so C shrinks and cores
    # stay balanced.
    import heapq

    ntile_tot = n_cores * ntiles
    deg_all = np.bincount(src, minlength=n_nodes)
    order_n = np.argsort(-deg_all, kind="stable")
    heap = [(0, t) for t in range(ntile_tot)]
    heapq.heapify(heap)
    fill = np.zeros(ntile_tot, dtype=np.int64)
    node_tile = np.zeros(n_nodes, dtype=np.int64)
    node_slot = np.zeros(n_nodes, dtype=np.int64)
    for n in order_n:
        while True:
            w, t = heapq.heappop(heap)
            if fill[t] < P:
                break
        node_tile[n] = t
        node_slot[n] = fill[t]
        fill[t] += 1
        if fill[t] < P:
            heapq.heappush(heap, (w + int(deg_all[n]), t))

    # group edges by src tile
    order = np.argsort(node_tile[src], kind="stable")
    src_s = src[order]
    dst_s = dst[order]
    gtile = node_tile[src_s]

    counts = np.bincount(gtile, minlength=ntile_tot)
    C = int(math.ceil(counts.max() / P))

    # deg-0 detection per (core, tile)
    deg0_nodes = np.nonzero(deg_all == 0)[0]
    deg0_tiles = tuple(
        sorted({(int(node_tile[n]) // ntiles, int(node_tile[n]) % ntiles)
                for n in deg0_nodes})
    )
    cfg = Cfg(n_nodes=n_nodes, n_cores=n_cores, C=C, deg0_tiles=deg0_tiles)
    slots_per_tile = C * P

    starts = np.zeros(ntile_tot, dtype=np.int64)
    starts[1:] = np.cumsum(counts)[:-1]
    slot_in_tile = np.arange(n_edges) - starts[gtile]

    # slot s of tile t -> (chunk c = s // P, partition p = s % P)
    flat = gtile * slots_per_tile + slot_in_tile
    dstI = np.zeros(ntile_tot * slots_per_tile, dtype=np.int64)
    valid = np.zeros(ntile_tot * slots_per_tile, dtype=bool)
    srcL = np.full(ntile_tot * slots_per_tile, -1.0, dtype=np.float32)
    e0 = np.zeros(ntile_tot * slots_per_tile, dtype=np.float32)
    e1 = np.zeros(ntile_tot * slots_per_tile, dtype=np.float32)
    dstI[flat] = dst_s
    valid[flat] = True
    srcL[flat] = node_slot[src_s]
    e0[flat] = st[src_s, 0] + st[dst_s, 2]
    e1[flat] = st[src_s, 1] + st[dst_s, 3]

    xq = x.astype(np.float16)
    # xT in LPT row order (for the deg-0 fallback Wh matmul)
    rowperm = np.zeros(ntile_tot * P, dtype=np.int64)
    rowperm[node_tile * P + node_slot] = np.arange(n_nodes)
    # pad rows (slots never filled) -> node 0, harmless
    filled = np.zeros(ntile_tot * P, dtype=bool)
    filled[node_tile * P + node_slot] = True
    rowperm[~filled] = 0
    xT = np.ascontiguousarray(xq[rowperm].T)  # [256, npad] f16

    wseq = W_w.astype(np.float16)  # [256, 256]
    iota = np.broadcast_to(
        np.arange(P, dtype=np.float16), (P, P)
    ).copy()  # iota[p, j] = j

    shared = {"xT": xT, "wseq": wseq, "iota": iota}
    per_core = []
    ctc = C * P
    for k in range(n_cores):
        sl = slice(k * ntiles * ctc, (k + 1) * ntiles * ctc)
        blk = np.zeros((ntiles, C, P, EW), dtype=np.float16)
        blk[:, :, :, 0:IN_DIM] = xq[dstI[sl].reshape(ntiles, C, P)]
        blk[:, :, :, 0:IN_DIM] *= valid[sl].reshape(ntiles, C, P)[..., None]
        blk[:, :, :, 256] = e0[sl].reshape(ntiles, C, P)
        blk[:, :, :, 257] = e1[sl].reshape(ntiles, C, P)
        blk[:, :, :, 258] = srcL[sl].reshape(ntiles, C, P)
        # partition-major: [tile, p, c, col]
        eblob = np.ascontiguousarray(blk.transpose(0, 2, 1, 3)).reshape(
            ntiles * P, C * EW
        )
        tl = slice(k * ntiles, (k + 1) * ntiles)
        mine = (node_tile >= k * ntiles) & (node_tile < (k + 1) * ntiles)
        nodes_k = np.nonzero(mine)[0]
        rows_k = (node_tile[nodes_k] - k * ntiles) * P + node_slot[nodes_k]
        per_core.append({"eblob": eblob, "_nodes": nodes_k, "_rows": rows_k})
    return cfg, shared, per_core


def build_program(cfg: Cfg, core_id: int = None):
    """Build the Bass/Tile program. Programs differ across cores only if
    deg0_tiles is non-empty; core_id selects which deg-0 gates to include
    (None = include none)."""
    C, ntiles, npad = cfg.C, cfg.ntiles, cfg.npad
    deg0 = {t for (c, t) in cfg.deg0_tiles if core_id is None or c == core_id}
    nc = bacc.Bacc("TRN2", target_bir_lowering=False, debug=False)

    xT_d = nc.dram_tensor("xT", [IN_DIM, npad], F16, kind="ExternalInput")
    w_d = nc.dram_tensor("wseq", [IN_DIM, 2 * OUT_DIM], F16, kind="ExternalInput")
    iota_d = nc.dram_tensor("iota", [P, P], F16, kind="ExternalInput")
    xe_d = nc.dram_tensor("eblob", [ntiles * P, C * EW], F16, kind="ExternalInput")
    out_d = nc.dram_tensor("out", [ntiles * P, 2 * OUT_DIM], F32, kind="ExternalOutput")

    OGRP = 8

    with tile.TileContext(nc) as tc:
        with (
            tc.tile_pool(name="const", bufs=1) as constp,
            tc.tile_pool(name="xe", bufs=3) as xep,
            tc.tile_pool(name="oneh", bufs=2) as onehp,
            tc.tile_pool(name="pon", bufs=2) as ponp,
            tc.tile_pool(name="sc", bufs=2) as scp,
            tc.tile_pool(name="zt_ps", bufs=1, space="PSUM") as ztps,
            tc.tile_pool(name="den_ps", bufs=2, space="PSUM") as denps,
            tc.tile_pool(name="out_ps", bufs=1, space="PSUM") as outps,
            tc.tile_pool(name="fin", bufs=3) as finp,
            tc.tile_pool(name="og", bufs=2) as ogp,
            tc.tile_pool(name="xo", bufs=2) as xop,
        ):
            # ---- constants ----
            wsb = constp.tile([P, 2, 2 * OUT_DIM], F16, tag="wsb")
            nc.sync.dma_start(
                out=wsb[:],
                in_=w_d[:, :].rearrange("(kt kp) c -> kp kt c", kp=P),
            )
            iota_t = constp.tile([P, P], F16, tag="iota")
            nc.sync.dma_start(out=iota_t[:], in_=iota_d[:, :])
            shift_t = constp.tile([P, 1], F32, tag="shift")
            nc.vector.memset(shift_t[:], -SHIFT)
            zero_t = constp.tile([P, 1], F32, tag="zero")
            nc.vector.memset(zero_t[:], 0.0)

            og = None
            for t in [tt for _ in range(cfg.reps) for tt in range(ntiles)]:
                g = t % OGRP
                if g == 0:
                    og = ogp.tile([P, OGRP, 2 * OUT_DIM], F32, tag="og")

                xe = xep.tile([P, C, EW], F16, tag="xe")
                nc.sync.dma_start(
                    out=xe[:].rearrange("p c w -> p (c w)"),
                    in_=xe_d[t * P : (t + 1) * P, :],
                )

                # one-hot of src slots: oneh[e, c, s] = (srcL[e, c] == s)
                oneh = onehp.tile([P, C, P], F16, tag="oneh")
                srcL_ap = bass.AP(
                    xe[:].tensor, xe[:].offset + 258,
                    [list(xe[:].ap[0]), [EW, C], [0, P]],
                )
                nc.vector.tensor_tensor(
                    out=oneh[:],
                    in0=srcL_ap,
                    in1=_ap_expand(iota_t[:], [(0, C), (1, P)]),
                    op=mybir.AluOpType.is_equal,
                )

                # p = exp(lrelu(e) - SHIFT), e = pre-added scores in cols 256:258
                e_ap = bass.AP(
                    xe[:].tensor, xe[:].offset + 256,
                    [list(xe[:].ap[0]), [EW, C], [1, 2]],
                )
                e_s = scp.tile([P, C, 2], F32, tag="e_s")
                nc.vector.tensor_scalar(
                    out=e_s[:], in0=e_ap, scalar1=0.2, scalar2=None,
                    op0=mybir.AluOpType.mult,
                )
                lr = scp.tile([P, C, 2], F32, tag="lr")
                nc.vector.tensor_tensor(
                    out=lr[:], in0=e_s[:], in1=e_ap, op=mybir.AluOpType.max,
                )
                p16 = scp.tile([P, C, 2], F16, tag="p16")
                nc.scalar.activation(
                    out=p16[:], in_=lr[:],
                    func=mybir.ActivationFunctionType.Exp,
                    bias=shift_t[:, 0:1],
                )

                # poneh_h = p_h * oneh  (head 0 on DVE, head 1 on Pool)
                pon = ponp.tile([P, 2, C, P], F16, tag="pon")
                for h, eng in ((0, nc.vector), (1, nc.vector)):
                    ph_ap = bass.AP(
                        p16[:].tensor, p16[:].offset + h,
                        [list(p16[:].ap[0]), [2, C], [0, P]],
                    )
                    eng.tensor_tensor(
                        out=pon[:, h, :, :], in0=oneh[:], in1=ph_ap,
                        op=mybir.AluOpType.mult,
                    )

                # zT[x, s] accumulation + denominators
                # (each matmul accumulation group needs its own PSUM tile)
                zt0 = ztps.tile([P, P], F32, tag="zt0")
                zt1 = ztps.tile([P, P], F32, tag="zt1")
                zt2 = ztps.tile([P, P], F32, tag="zt2")
                zt3 = ztps.tile([P, P], F32, tag="zt3")
                ztl = [zt0, zt1, zt2, zt3]
                den = denps.tile([P, 2], F32, tag="den")
                for c in range(C):
                    st = (c == 0)
                    sp = (c == C - 1)
                    for h in range(2):
                        for kx in range(2):
                            nc.tensor.matmul(
                                out=ztl[2 * h + kx][:],
                                lhsT=xe[:, c, kx * P : (kx + 1) * P],
                                rhs=pon[:, h, c, :],
                                start=st, stop=sp,
                            )
                    nc.tensor.matmul(
                        out=den[:],
                        lhsT=oneh[:, c, :],
                        rhs=p16[:, c, :],
                        start=st, stop=sp,
                    )

                # znT: PSUM -> SBUF f16 (Act engine)
                znT = finp.tile([P, 4, P], F16, tag="znT")
                for zg in range(4):
                    nc.scalar.activation(
                        out=znT[:, zg, :], in_=ztl[zg][:],
                        func=mybir.ActivationFunctionType.Copy,
                    )

                # out[s, :] = zT @ W  (per head, 2 x-chunks)
                ops0 = outps.tile([P, OUT_DIM], F32, tag="ops0")
                ops1 = outps.tile([P, OUT_DIM], F32, tag="ops1")
                opsl = [ops0, ops1]
                for h in range(2):
                    for kx in range(2):
                        nc.tensor.matmul(
                            out=opsl[h][:],
                            lhsT=znT[:, 2 * h + kx, :],
                            rhs=wsb[:, kx, h * OUT_DIM : (h + 1) * OUT_DIM],
                            start=(kx == 0), stop=(kx == 1),
                        )

                # normalize by 1/den during copy-out
                dns = finp.tile([P, 2], F32, tag="dns")
                nc.vector.tensor_scalar(
                    out=dns[:], in0=den[:], scalar1=1e-30, scalar2=None,
                    op0=mybir.AluOpType.max,
                )
                rcp = finp.tile([P, 2], F32, tag="rcp")
                nc.vector.reciprocal(out=rcp[:], in_=dns[:])
                for h in range(2):
                    nc.vector.tensor_scalar(
                        out=og[:, g, h * OUT_DIM : (h + 1) * OUT_DIM],
                        in0=opsl[h][:],
                        scalar1=rcp[:, h : h + 1],
                        scalar2=None,
                        op0=mybir.AluOpType.mult,
                    )

                if t in deg0:
                    # deg-0 rows: out = Wh[own] where den == 0
                    xo = xop.tile([P, 2, P], F16, tag="xo")
                    nc.sync.dma_start(
                        out=xo[:],
                        in_=xT_d[:, t * P : (t + 1) * P].rearrange(
                            "(kt kp) e -> kp kt e", kp=P
                        ),
                    )
                    fb = outps.tile([P, 2 * OUT_DIM], F32, tag="fb")
                    for kt in range(2):
                        nc.tensor.matmul(
                            out=fb[:],
                            lhsT=xo[:, kt, :],
                            rhs=wsb[:, kt, :],
                            start=(kt == 0), stop=(kt == 1),
                        )
                    nmask = finp.tile([P, 1], F32, tag="nmask")
                    nc.vector.tensor_scalar(
                        out=nmask[:], in0=den[:, 0:1], scalar1=0.0,
                        scalar2=None, op0=mybir.AluOpType.is_le,
                    )
                    fbm = finp.tile([P, 2 * OUT_DIM], F32, tag="fbm")
                    nc.vector.tensor_scalar(
                        out=fbm[:], in0=fb[:], scalar1=nmask[:, 0:1],
                        scalar2=None, op0=mybir.AluOpType.mult,
                    )
                    nc.vector.tensor_tensor(
                        out=og[:, g, :], in0=og[:, g, :], in1=fbm[:],
                        op=mybir.AluOpType.add,
                    )

                if g == OGRP - 1 or t == ntiles - 1:
                    t0 = t - g
                    nc.sync.dma_start(
                        out=out_d[t0 * P : (t + 1) * P, :].rearrange(
                            "(g p) c -> p g c", p=P
                        ),
                        in_=og[:, 0 : g + 1, :],
                    )

    nc.compile()
    return nc


_prog_cache = {}


def kernel(x, edge_index, W_w, W_b, a):
    cfg, shared, per_core = host_prep(x, edge_index, W_w, W_b, a, n_cores=8)
    key = (cfg, None) if not cfg.deg0_tiles else (cfg, "multi")
    if not cfg.deg0_tiles:
        if key not in _prog_cache:
            _prog_cache[key] = build_program(cfg)
        ncs = [_prog_cache[key]] * cfg.n_cores
    else:
        # per-core programs when deg-0 gating differs
        ncs = []
        for k in range(cfg.n_cores):
            kk = (cfg, k)
            if kk not in _prog_cache:
                _prog_cache[kk] = build_program(cfg, core_id=k)
            ncs.append(_prog_cache[kk])

    out = np.zeros((cfg.n_nodes, 2 * OUT_DIM), dtype=np.float32)
    if not cfg.deg0_tiles:
        in_maps = [
            {kk: v for kk, v in {**shared, **pc}.items() if not kk.startswith("_")}
            for pc in per_core
        ]
        res = run_bass_kernel_spmd(ncs[0], in_maps, list(range(cfg.n_cores)))
        for k in range(cfg.n_cores):
            pc = per_core[k]
            out[pc["_nodes"]] = res.results[k]["out"][pc["_rows"]]
    else:
        # different programs per core: run one core at a time (rare path)
        for k in range(cfg.n_cores):
            pc = per_core[k]
            in_map = {
                kk: v for kk, v in {**shared, **pc}.items()
                if not kk.startswith("_")
            }
            res = run_bass_kernel_spmd(ncs[k], [in_map], [k])
            out[pc["_nodes"]] = res.results[k - k]["out"][pc["_rows"]]
    return out
